# revision 1
# baseline (speedup 1.0000x reference)
"""BERT self-attention (B=4, S=2048, D=1024, H=16) on 8 trn2 NeuronCores.

Sharding: core c -> (batch b = c//2, head-group hg = c%2, 8 heads each).
Each core computes out[b, :, hg*512:(hg+1)*512] independently; host
gathers. Inputs are pre-transposed on host so the contraction dim (d)
lands on SBUF partitions: xt = X.T [D,S], w{q,k,v}t = W.T shard [D,512].

v2 design (ACT-bound pipeline, ~all engines overlapped):
  - Q^T/K^T pair-tiles [128, S] f32r (2 heads per tile, dh on partitions).
  - V_aug [128j, 8h, 65] bf16 per s-tile: V + bias, col 64 = ones (gives
    the softmax denominator for free during the ctx matmul).
  - Attention per (pair p, query-quarter qc): 16 j-tiles; scores for the
    2 heads go to one [128, 2, 512] PSUM tile (2 banks) via concurrent
    row-group matmuls; ONE exp per jt ([128,1024] ACT op, mask as bias);
    ctx accumulated IN PSUM across all 16 jt (C[65,512] per head) - no
    DVE adds in the inner loop.
  - Drain: C -> SBUF copy, PE-transpose 128-blocks, reciprocal of the
    denominator row, scale, DMA out.
  - V and all QK projections run contiguously up front (interleaving
    proj chunks into attention measured worse: aux-slot contention
    inflates proj matmuls 247->403ns and triples exp gaps).
PSUM budget: sp0(2) + sp1(2) + c0(1) + c1(1) + aux(2) = 8 banks.
"""

import numpy as np

import concourse.bass as bass
import concourse.tile as tile
from concourse import bacc, mybir
from concourse.bass_utils import run_bass_kernel_spmd
from concourse.masks import make_identity

B, S, D, H = 4, 2048, 1024, 16
DH = 64
O = 512  # per-core output width (8 heads)
HL = 8  # local heads per core
NP = 4  # head pairs per core
ST = S // 128  # 16 s-tiles
QC = 4  # query quarters (512 queries each)
F32 = mybir.dt.float32
F32R = mybir.dt.float32r
BF16 = mybir.dt.bfloat16
EXP = mybir.ActivationFunctionType.Exp

_NC_CACHE = None


def build_nc():
    nc = bacc.Bacc(
        "TRN2",
        target_bir_lowering=False,
        debug=False,
        enable_asserts=True,
        num_devices=8,
    )
    xt = nc.dram_tensor("xt", [D, S], F32R, kind="ExternalInput").ap()
    wqt = nc.dram_tensor("wqt", [D, O], F32R, kind="ExternalInput").ap()
    wkt = nc.dram_tensor("wkt", [D, O], F32R, kind="ExternalInput").ap()
    wvt = nc.dram_tensor("wvt", [D, O], F32R, kind="ExternalInput").ap()
    bq = nc.dram_tensor("bq", [O], F32, kind="ExternalInput").ap()
    bk = nc.dram_tensor("bk", [O], F32, kind="ExternalInput").ap()
    bv = nc.dram_tensor("bv", [O], F32, kind="ExternalInput").ap()
    mask = nc.dram_tensor("mask", [S], F32, kind="ExternalInput").ap()
    out = nc.dram_tensor("out", [S, O], F32, kind="ExternalOutput").ap()

    with tile.TileContext(nc) as tc:
        _emit(nc, tc, xt, wqt, wkt, wvt, bq, bk, bv, mask, out)
    nc.compile()
    return nc


def _emit(nc, tc, xt, wqt, wkt, wvt, bq, bk, bv, mask, out):
    with (
        tc.tile_pool(name="singles", bufs=1) as singles,
        tc.tile_pool(name="persist", bufs=1) as persist,
        tc.tile_pool(name="wpool", bufs=1) as wpool,
        tc.tile_pool(name="attn", bufs=1) as attn,
        tc.tile_pool(name="psum", bufs=1, space="PSUM") as psum,
    ):
        ident = singles.tile([128, 128], F32)
        make_identity(nc, ident)
        mask_sb = singles.tile([128, ST], F32)
        nc.sync.dma_start(out=mask_sb, in_=mask.rearrange("(t p) -> p t", p=128))
        bq_sb = singles.tile([128, NP], F32)
        nc.sync.dma_start(out=bq_sb, in_=bq.rearrange("(t p) -> p t", p=128))
        bk_sb = singles.tile([128, NP], F32)
        nc.sync.dma_start(out=bk_sb, in_=bk.rearrange("(t p) -> p t", p=128))
        bv_bc = singles.tile([128, HL, DH], F32)
        nc.sync.dma_start(
            out=bv_bc, in_=bass.AP(tensor=bv.tensor, offset=0, ap=[[0, 128], [1, O]])
        )

        # persistent activations
        xts = [persist.tile([128, S], F32R, name=f"xts{dt}", tag=f"xts{dt}") for dt in range(8)]
        qts = [persist.tile([128, S], F32R, name=f"qt{p}", tag=f"qt{p}") for p in range(NP)]
        kts = [persist.tile([128, S], F32R, name=f"kt{p}", tag=f"kt{p}") for p in range(NP)]
        vaug = [
            persist.tile([128, HL, DH + 1], BF16, name=f"vaug{t}", tag=f"vaug{t}")
            for t in range(ST)
        ]

        for dt in range(8):
            nc.sync.dma_start(out=xts[dt], in_=xt[dt * 128 : (dt + 1) * 128, :])

        # ---- V projection (all heads, up front) ----
        wv_t = []
        for dt in range(8):
            w = wpool.tile([128, O], F32R, name=f"wv{dt}", tag="wv", bufs=8)
            nc.sync.dma_start(out=w, in_=wvt[dt * 128 : (dt + 1) * 128, :])
            wv_t.append(w)
        for st in range(ST):
            ps = psum.tile([128, HL, DH], F32, name=f"psv{st}", tag="aux", bufs=2)
            for dt in range(8):
                nc.tensor.matmul(
                    ps,
                    xts[dt][:, st * 128 : (st + 1) * 128],
                    wv_t[dt],
                    start=(dt == 0),
                    stop=(dt == 7),
                )
            va = vaug[st]
            nc.vector.memset(va[:, :, DH : DH + 1], 1.0)
            nc.vector.tensor_add(va[:, :, 0:DH], ps, bv_bc)

        # ---- Q/K projection machinery (per-pair, chunked) ----
        wslices = {}  # (which, p) -> list of 8 [128,128] tiles

        def load_w_slices(which, p):
            wdram = {"k": wkt, "q": wqt}[which]
            tiles = []
            for dt in range(8):
                w = wpool.tile(
                    [128, 128], F32R, name=f"w{which}{p}_{dt}", tag=f"w{which}", bufs=8
                )
                nc.sync.dma_start(
                    out=w,
                    in_=wdram[dt * 128 : (dt + 1) * 128, p * 128 : (p + 1) * 128],
                )
                tiles.append(w)
            wslices[which, p] = tiles

        def emit_qk_chunk(which, p, c):
            if (which, p) not in wslices:
                load_w_slices(which, p)
            wts = wslices[which, p]
            dst = {"k": kts, "q": qts}[which][p]
            bias_sb = {"k": bk_sb, "q": bq_sb}[which]
            ps = psum.tile([128, 512], F32, name=f"ps{which}{p}_{c}", tag="aux", bufs=2)
            for dt in range(8):
                nc.tensor.matmul(
                    ps,
                    wts[dt],
                    xts[dt][:, c * 512 : (c + 1) * 512],
                    start=(dt == 0),
                    stop=(dt == 7),
                )
            nc.vector.tensor_scalar_add(
                dst[:, c * 512 : (c + 1) * 512], ps, bias_sb[:, p : p + 1]
            )

        # All QK projections up front, contiguously: interleaving them into
        # attention's PE slack measured WORSE (proj matmuls inflate 247->403ns
        # on aux-slot contention and exp gaps triple in chunked blocks).
        for p in range(NP):
            for which in ("k", "q"):
                for c in range(4):
                    emit_qk_chunk(which, p, c)

        # ---- attention ----
        for p in range(NP):
            ktp, qtp = kts[p], qts[p]
            for qc in range(QC):
                base = qc * 512
                C = [
                    psum.tile(
                        [DH + 1, 512], F32, name=f"c{x}_{p}_{qc}", tag=f"c{x}", bufs=1
                    )
                    for x in range(2)
                ]
                for jt in range(ST):
                    sp = psum.tile(
                        [128, 2, 512],
                        F32,
                        name=f"sp{p}_{qc}_{jt}",
                        tag=f"sp{jt % 2}",
                        bufs=1,
                    )
                    for x in range(2):
                        hp = slice(x * 64, x * 64 + 64)
                        nc.tensor.matmul(
                            sp[:, x, :],
                            ktp[hp, jt * 128 : (jt + 1) * 128],
                            qtp[hp, base : base + 512],
                            start=True,
                            stop=True,
                        )
                    u = attn.tile(
                        [128, 2, 512],
                        BF16,
                        name=f"u{p}_{qc}_{jt}",
                        tag=f"u{jt % 2}",
                        bufs=1,
                    )
                    nc.scalar.activation(
                        u, sp, EXP, bias=mask_sb[:, jt : jt + 1], scale=0.125
                    )
                    for x in range(2):
                        nc.tensor.matmul(
                            C[x],
                            vaug[jt][:, 2 * p + x, :],
                            u[:, x, :],
                            start=(jt == 0),
                            stop=(jt == ST - 1),
                        )
                # drain: copy to SBUF, PE-transpose 128-blocks, then
                # normalize by the denominator row and store.
                for x in range(2):
                    hh = 2 * p + x
                    csb = attn.tile(
                        [DH + 1, 512], F32, name=f"csb{p}_{qc}_{x}", tag=f"csb{x}",
                        bufs=2,
                    )
                    nc.vector.tensor_copy(out=csb, in_=C[x])
                    for it in range(4):
                        tp_ = psum.tile(
                            [128, DH + 1], F32, name=f"tp{p}_{qc}_{x}_{it}",
                            tag="aux", bufs=2,
                        )
                        nc.tensor.transpose(
                            tp_,
                            csb[:, it * 128 : (it + 1) * 128],
                            ident[0 : DH + 1, 0 : DH + 1],
                        )
                        rc = attn.tile(
                            [128, 1], F32, name=f"rc{p}_{qc}_{x}_{it}", tag="rc", bufs=6
                        )
                        nc.vector.reciprocal(rc, tp_[:, DH : DH + 1])
                        ot = attn.tile(
                            [128, DH], F32, name=f"ot{p}_{qc}_{x}_{it}", tag="ot", bufs=6
                        )
                        nc.vector.tensor_scalar_mul(ot, tp_[:, 0:DH], rc)
                        row = base + it * 128
                        nc.sync.dma_start(
                            out=out[row : row + 128, hh * DH : (hh + 1) * DH], in_=ot
                        )


def _make_in_maps(hidden_states, attention_mask, Wq, bq, Wk, bk, Wv, bv):
    in_maps = []
    for c in range(8):
        b, hg = divmod(c, 2)
        sl = slice(hg * O, (hg + 1) * O)
        in_maps.append(
            {
                "xt": np.ascontiguousarray(hidden_states[b].T),
                "wqt": np.ascontiguousarray(Wq[sl, :].T),
                "wkt": np.ascontiguousarray(Wk[sl, :].T),
                "wvt": np.ascontiguousarray(Wv[sl, :].T),
                "bq": np.ascontiguousarray(bq[sl]),
                "bk": np.ascontiguousarray(bk[sl]),
                "bv": np.ascontiguousarray(bv[sl]),
                "mask": np.ascontiguousarray(attention_mask[b, 0, 0, :]),
            }
        )
    return in_maps


def _gather(results):
    out = np.empty((B, S, D), dtype=np.float32)
    for c in range(8):
        b, hg = divmod(c, 2)
        out[b, :, hg * O : (hg + 1) * O] = results[c]["out"]
    return out


def kernel(hidden_states, attention_mask, Wq, bq, Wk, bk, Wv, bv, **run_kwargs):
    global _NC_CACHE
    args = [hidden_states, attention_mask, Wq, bq, Wk, bk, Wv, bv]
    args = [np.asarray(a, dtype=np.float32) for a in args]
    if _NC_CACHE is None:
        _NC_CACHE = build_nc()
    in_maps = _make_in_maps(*args)
    res = run_bass_kernel_spmd(_NC_CACHE, in_maps, core_ids=list(range(8)), **run_kwargs)
    kernel.last_result = res
    return _gather(res.results)



# revision 2
# speedup vs baseline: 1.0667x; 1.0667x over previous
"""BERT self-attention (B=4, S=2048, D=1024, H=16) on 8 trn2 NeuronCores.

Sharding: core c -> (batch b = c//2, head-group hg = c%2, 8 heads each).
Each core computes out[b, :, hg*512:(hg+1)*512] independently; host
gathers. Inputs are pre-transposed on host so the contraction dim (d)
lands on SBUF partitions: xt = X.T [D,S], w{q,k,v}t = W.T shard [D,512].

v3 design (all-bf16 matmuls; PE+ACT co-bound):
  - Host converts xt and weight shards to bf16: halves input DMA and
    makes every matmul 1 cyc/row (f32r measured ~1.55 cyc/row on HW).
  - Q^T/K^T pair-tiles [128, S] bf16 (2 heads per tile, dh on partitions).
  - V_aug [128j, 8h, 65] bf16 per s-tile: V + bias, col 64 = ones (gives
    the softmax denominator for free during the ctx matmul).
  - Attention per (pair p, query-quarter qc): 16 j-tiles; scores for the
    2 heads go to one [128, 2, 512] PSUM tile (2 banks) via row-group
    matmuls; ONE exp per jt ([128,1024] ACT op, mask as bias);
    ctx accumulated IN PSUM across all 16 jt (C[65,512] per head).
  - Drain: C -> SBUF copy, PE-transpose 128-blocks, reciprocal of the
    denominator row, scale into a staging tile, ONE batched DMA out per
    (p, qc) ([128,4,2,64] -> 512 rows x 128 cols).
PSUM budget: sp0(2) + sp1(2) + c0(1) + c1(1) + aux(2) = 8 banks.
"""

import ml_dtypes
import numpy as np

import concourse.bass as bass
import concourse.tile as tile
from concourse import bacc, mybir
from concourse.bass_utils import run_bass_kernel_spmd
from concourse.masks import make_identity

B, S, D, H = 4, 2048, 1024, 16
DH = 64
O = 512  # per-core output width (8 heads)
HL = 8  # local heads per core
NP = 4  # head pairs per core
ST = S // 128  # 16 s-tiles
QC = 4  # query quarters (512 queries each)
F32 = mybir.dt.float32
BF16 = mybir.dt.bfloat16
EXP = mybir.ActivationFunctionType.Exp

_NC_CACHE = None


def build_nc():
    nc = bacc.Bacc(
        "TRN2",
        target_bir_lowering=False,
        debug=False,
        enable_asserts=True,
        num_devices=8,
    )
    xt = nc.dram_tensor("xt", [D, S], BF16, kind="ExternalInput").ap()
    wqt = nc.dram_tensor("wqt", [D, O], BF16, kind="ExternalInput").ap()
    wkt = nc.dram_tensor("wkt", [D, O], BF16, kind="ExternalInput").ap()
    wvt = nc.dram_tensor("wvt", [D, O], BF16, kind="ExternalInput").ap()
    bq = nc.dram_tensor("bq", [O], F32, kind="ExternalInput").ap()
    bk = nc.dram_tensor("bk", [O], F32, kind="ExternalInput").ap()
    bv = nc.dram_tensor("bv", [O], F32, kind="ExternalInput").ap()
    mask = nc.dram_tensor("mask", [S], F32, kind="ExternalInput").ap()
    out = nc.dram_tensor("out", [S, O], F32, kind="ExternalOutput").ap()

    with tile.TileContext(nc) as tc:
        _emit(nc, tc, xt, wqt, wkt, wvt, bq, bk, bv, mask, out)
    nc.compile()
    return nc


def _emit(nc, tc, xt, wqt, wkt, wvt, bq, bk, bv, mask, out):
    with (
        tc.tile_pool(name="singles", bufs=1) as singles,
        tc.tile_pool(name="persist", bufs=1) as persist,
        tc.tile_pool(name="wpool", bufs=1) as wpool,
        tc.tile_pool(name="attn", bufs=1) as attn,
        tc.tile_pool(name="psum", bufs=1, space="PSUM") as psum,
    ):
        ident = singles.tile([128, 128], F32)
        make_identity(nc, ident)
        mask_sb = singles.tile([128, ST], F32)
        nc.sync.dma_start(out=mask_sb, in_=mask.rearrange("(t p) -> p t", p=128))
        bq_sb = singles.tile([128, NP], F32)
        nc.sync.dma_start(out=bq_sb, in_=bq.rearrange("(t p) -> p t", p=128))
        bk_sb = singles.tile([128, NP], F32)
        nc.sync.dma_start(out=bk_sb, in_=bk.rearrange("(t p) -> p t", p=128))
        bv_bc = singles.tile([128, HL, DH], F32)
        nc.sync.dma_start(
            out=bv_bc, in_=bass.AP(tensor=bv.tensor, offset=0, ap=[[0, 128], [1, O]])
        )

        # persistent activations (all bf16)
        xts = [persist.tile([128, S], BF16, name=f"xts{dt}", tag=f"xts{dt}") for dt in range(8)]
        qts = [persist.tile([128, S], BF16, name=f"qt{p}", tag=f"qt{p}") for p in range(NP)]
        kts = [persist.tile([128, S], BF16, name=f"kt{p}", tag=f"kt{p}") for p in range(NP)]
        vaug = [
            persist.tile([128, HL, DH + 1], BF16, name=f"vaug{t}", tag=f"vaug{t}")
            for t in range(ST)
        ]

        # V weights first so V-proj is gated only on xt arrival
        wv_t = []
        for dt in range(8):
            w = wpool.tile([128, O], BF16, name=f"wv{dt}", tag="wv", bufs=8)
            nc.sync.dma_start(out=w, in_=wvt[dt * 128 : (dt + 1) * 128, :])
            wv_t.append(w)
        for dt in range(8):
            nc.sync.dma_start(out=xts[dt], in_=xt[dt * 128 : (dt + 1) * 128, :])

        # ---- V projection (all heads, up front) ----
        for st in range(ST):
            ps = psum.tile([128, HL, DH], F32, name=f"psv{st}", tag="aux", bufs=2)
            for dt in range(8):
                nc.tensor.matmul(
                    ps,
                    xts[dt][:, st * 128 : (st + 1) * 128],
                    wv_t[dt],
                    start=(dt == 0),
                    stop=(dt == 7),
                )
            va = vaug[st]
            nc.vector.memset(va[:, :, DH : DH + 1], 1.0)
            nc.vector.tensor_add(va[:, :, 0:DH], ps, bv_bc)

        # ---- Q/K projection machinery (per-pair, chunked) ----
        wslices = {}  # (which, p) -> list of 8 [128,128] tiles

        def load_w_slices(which, p):
            wdram = {"k": wkt, "q": wqt}[which]
            tiles = []
            for dt in range(8):
                w = wpool.tile(
                    [128, 128], BF16, name=f"w{which}{p}_{dt}", tag=f"w{which}", bufs=8
                )
                nc.sync.dma_start(
                    out=w,
                    in_=wdram[dt * 128 : (dt + 1) * 128, p * 128 : (p + 1) * 128],
                )
                tiles.append(w)
            wslices[which, p] = tiles

        def emit_qk_chunk(which, p, c):
            if (which, p) not in wslices:
                load_w_slices(which, p)
            wts = wslices[which, p]
            dst = {"k": kts, "q": qts}[which][p]
            bias_sb = {"k": bk_sb, "q": bq_sb}[which]
            ps = psum.tile([128, 512], F32, name=f"ps{which}{p}_{c}", tag="aux", bufs=2)
            for dt in range(8):
                nc.tensor.matmul(
                    ps,
                    wts[dt],
                    xts[dt][:, c * 512 : (c + 1) * 512],
                    start=(dt == 0),
                    stop=(dt == 7),
                )
            nc.vector.tensor_scalar_add(
                dst[:, c * 512 : (c + 1) * 512], ps, bias_sb[:, p : p + 1]
            )

        for p in range(NP):
            for which in ("k", "q"):
                for c in range(4):
                    emit_qk_chunk(which, p, c)

        # ---- attention ----
        for p in range(NP):
            ktp, qtp = kts[p], qts[p]
            for qc in range(QC):
                base = qc * 512
                C = [
                    psum.tile(
                        [DH + 1, 512], F32, name=f"c{x}_{p}_{qc}", tag=f"c{x}", bufs=1
                    )
                    for x in range(2)
                ]
                for jt in range(ST):
                    sp = psum.tile(
                        [128, 2, 512],
                        F32,
                        name=f"sp{p}_{qc}_{jt}",
                        tag=f"sp{jt % 2}",
                        bufs=1,
                    )
                    for x in range(2):
                        hp = slice(x * 64, x * 64 + 64)
                        nc.tensor.matmul(
                            sp[:, x, :],
                            ktp[hp, jt * 128 : (jt + 1) * 128],
                            qtp[hp, base : base + 512],
                            start=True,
                            stop=True,
                        )
                    u = attn.tile(
                        [128, 2, 512],
                        BF16,
                        name=f"u{p}_{qc}_{jt}",
                        tag=f"u{jt % 2}",
                        bufs=1,
                    )
                    nc.scalar.activation(
                        u, sp, EXP, bias=mask_sb[:, jt : jt + 1], scale=0.125
                    )
                    for x in range(2):
                        nc.tensor.matmul(
                            C[x],
                            vaug[jt][:, 2 * p + x, :],
                            u[:, x, :],
                            start=(jt == 0),
                            stop=(jt == ST - 1),
                        )
                # drain: copy to SBUF, PE-transpose 128-blocks, then
                # normalize by the denominator row into a staging tile,
                # and store with ONE batched DMA per (p, qc).
                ot = attn.tile([128, 4, 2, DH], F32, name=f"ot{p}_{qc}", tag="ot", bufs=2)
                csb = []
                for x in range(2):
                    cs = attn.tile(
                        [DH + 1, 512], F32, name=f"csb{p}_{qc}_{x}", tag=f"csb{x}",
                        bufs=2,
                    )
                    nc.vector.tensor_copy(out=cs, in_=C[x])
                    csb.append(cs)
                for it in range(4):
                    for x in range(2):
                        tp_ = psum.tile(
                            [128, DH + 1], F32, name=f"tp{p}_{qc}_{x}_{it}",
                            tag="aux", bufs=2,
                        )
                        nc.tensor.transpose(
                            tp_,
                            csb[x][:, it * 128 : (it + 1) * 128],
                            ident[0 : DH + 1, 0 : DH + 1],
                        )
                        rc = attn.tile(
                            [128, 1], F32, name=f"rc{p}_{qc}_{x}_{it}", tag="rc", bufs=6
                        )
                        nc.vector.reciprocal(rc, tp_[:, DH : DH + 1])
                        nc.vector.tensor_scalar_mul(ot[:, it, x, :], tp_[:, 0:DH], rc)
                nc.sync.dma_start(
                    out=out[base : base + 512, 2 * p * DH : (2 * p + 2) * DH].rearrange(
                        "(i p) c -> p i c", p=128
                    ),
                    in_=ot,
                )


def _make_in_maps(hidden_states, attention_mask, Wq, bq, Wk, bk, Wv, bv):
    bf = ml_dtypes.bfloat16
    in_maps = []
    for c in range(8):
        b, hg = divmod(c, 2)
        sl = slice(hg * O, (hg + 1) * O)
        in_maps.append(
            {
                "xt": np.ascontiguousarray(hidden_states[b].T.astype(bf)),
                "wqt": np.ascontiguousarray(Wq[sl, :].T.astype(bf)),
                "wkt": np.ascontiguousarray(Wk[sl, :].T.astype(bf)),
                "wvt": np.ascontiguousarray(Wv[sl, :].T.astype(bf)),
                "bq": np.ascontiguousarray(bq[sl]),
                "bk": np.ascontiguousarray(bk[sl]),
                "bv": np.ascontiguousarray(bv[sl]),
                "mask": np.ascontiguousarray(attention_mask[b, 0, 0, :]),
            }
        )
    return in_maps


def _gather(results):
    out = np.empty((B, S, D), dtype=np.float32)
    for c in range(8):
        b, hg = divmod(c, 2)
        out[b, :, hg * O : (hg + 1) * O] = results[c]["out"]
    return out


def kernel(hidden_states, attention_mask, Wq, bq, Wk, bk, Wv, bv, **run_kwargs):
    global _NC_CACHE
    args = [hidden_states, attention_mask, Wq, bq, Wk, bk, Wv, bv]
    args = [np.asarray(a, dtype=np.float32) for a in args]
    if _NC_CACHE is None:
        _NC_CACHE = build_nc()
    in_maps = _make_in_maps(*args)
    res = run_bass_kernel_spmd(_NC_CACHE, in_maps, core_ids=list(range(8)), **run_kwargs)
    kernel.last_result = res
    return _gather(res.results)


# revision 4
# speedup vs baseline: 1.2359x; 1.1586x over previous
"""BERT self-attention (B=4, S=2048, D=1024, H=16) on 8 trn2 NeuronCores.

Sharding: core c -> (batch b = c//2, head-group hg = c%2, 8 heads each).
Each core computes out[b, :, hg*512:(hg+1)*512] independently; host
gathers. Inputs are pre-transposed on host so the contraction dim (d)
lands on SBUF partitions: xt = X.T [D,S], w{q,k,v}t = W.T shard [D,512].

v4 design (all-bf16, fully software-pipelined, ACT(exp)-paced):
  - Host converts xt/weights to bf16 (halves DMA; every matmul 1 cyc/row).
  - V-projection raced against the xt DMA: dt-outer over ALL 8 PSUM
    banks (2 passes x 8 s-tiles), so PE consumes xt chunks as they land.
  - QK projection: kt(p0)+qt(p0,c0) up front; every remaining chunk is
    drip-fed into the attention jt-slots (proj matmuls fill the PE slack
    while ACT runs exp), paced per-pair so pair p+1's Q/K finish during
    pair p's attention.
  - Attention per (pair, query-quarter): scores for 2 heads as
    concurrent row-group matmuls -> [128,2,512] PSUM; ONE exp per jt
    ([128,1024] ACT op, mask as bias); ctx accumulates in PSUM.
    ctx LAGS TWO SLOTS behind scores/exp (sp bufs=2, u bufs=3): the PE
    never head-of-line blocks on exp, and at qc boundaries the C-bank
    drain (DVE copy) gets 2 slots of slack before ctx(start=True) reuses
    the bank.
  - Drain: C -> SBUF copy, then 8 (transpose, reciprocal, scale) steps
    interleaved one-per-slot into the next qc, one batched DMA out.
PSUM: sp0(2) + sp1(2) + c0(1) + c1(1) + proj(1) + tp(1) = 8 banks.
"""

import ml_dtypes
import numpy as np

import concourse.bass as bass
import concourse.tile as tile
from concourse import bacc, mybir
from concourse.bass_utils import run_bass_kernel_spmd
from concourse.masks import make_identity

B, S, D, H = 4, 2048, 1024, 16
DH = 64
O = 512  # per-core output width (8 heads)
HL = 8  # local heads per core
NP = 4  # head pairs per core
ST = S // 128  # 16 s-tiles
QC = 4  # query quarters (512 queries each)
F32 = mybir.dt.float32
BF16 = mybir.dt.bfloat16
EXP = mybir.ActivationFunctionType.Exp

_NC_CACHE = None


def build_nc():
    nc = bacc.Bacc(
        "TRN2",
        target_bir_lowering=False,
        debug=False,
        enable_asserts=True,
        num_devices=8,
    )
    xt = nc.dram_tensor("xt", [D, S], BF16, kind="ExternalInput").ap()
    wqt = nc.dram_tensor("wqt", [D, O], BF16, kind="ExternalInput").ap()
    wkt = nc.dram_tensor("wkt", [D, O], BF16, kind="ExternalInput").ap()
    wvt = nc.dram_tensor("wvt", [D, O], BF16, kind="ExternalInput").ap()
    bq = nc.dram_tensor("bq", [O], F32, kind="ExternalInput").ap()
    bk = nc.dram_tensor("bk", [O], F32, kind="ExternalInput").ap()
    bv = nc.dram_tensor("bv", [O], F32, kind="ExternalInput").ap()
    mask = nc.dram_tensor("mask", [S], F32, kind="ExternalInput").ap()
    out = nc.dram_tensor("out", [S, O], F32, kind="ExternalOutput").ap()

    with tile.TileContext(nc) as tc:
        _emit(nc, tc, xt, wqt, wkt, wvt, bq, bk, bv, mask, out)
    nc.compile()
    return nc


def _emit(nc, tc, xt, wqt, wkt, wvt, bq, bk, bv, mask, out):
    with (
        tc.tile_pool(name="singles", bufs=1) as singles,
        tc.tile_pool(name="persist", bufs=1) as persist,
        tc.tile_pool(name="wpool", bufs=1) as wpool,
        tc.tile_pool(name="attn", bufs=1) as attn,
        tc.tile_pool(name="psum", bufs=1, space="PSUM") as psum,
    ):
        ident = singles.tile([128, 128], F32)
        make_identity(nc, ident)
        mask_sb = singles.tile([128, ST], F32)
        nc.sync.dma_start(out=mask_sb, in_=mask.rearrange("(t p) -> p t", p=128))
        bq_sb = singles.tile([128, NP], F32)
        nc.sync.dma_start(out=bq_sb, in_=bq.rearrange("(t p) -> p t", p=128))
        bk_sb = singles.tile([128, NP], F32)
        nc.sync.dma_start(out=bk_sb, in_=bk.rearrange("(t p) -> p t", p=128))
        bv_bc = singles.tile([128, HL, DH], F32)
        nc.sync.dma_start(
            out=bv_bc, in_=bass.AP(tensor=bv.tensor, offset=0, ap=[[0, 128], [1, O]])
        )

        # persistent activations (all bf16)
        xts = [persist.tile([128, S], BF16, name=f"xts{dt}", tag=f"xts{dt}") for dt in range(8)]
        qts = [persist.tile([128, S], BF16, name=f"qt{p}", tag=f"qt{p}") for p in range(NP)]
        kts = [persist.tile([128, S], BF16, name=f"kt{p}", tag=f"kt{p}") for p in range(NP)]
        vaug = [
            persist.tile([128, HL, DH + 1], BF16, name=f"vaug{t}", tag=f"vaug{t}")
            for t in range(ST)
        ]

        # V weights FIRST on the DMA queue (V proj races the xt DMA),
        # then xt, then all QK weight slices (one batched DMA each).
        wv = wpool.tile([128, 8, O], BF16, name="wv", tag="wv")
        nc.sync.dma_start(
            out=wv, in_=wvt.rearrange("(t p) o -> p t o", p=128)
        )
        for dt in range(8):
            nc.sync.dma_start(out=xts[dt], in_=xt[dt * 128 : (dt + 1) * 128, :])
        wsl = {}  # (which, p) -> [128, 8, 128] tile, dt on free dim
        for which, wdram in (("k", wkt), ("q", wqt)):
            for p in range(NP):
                w = wpool.tile(
                    [128, 8, 128], BF16, name=f"w{which}{p}", tag=f"w{which}{p}"
                )
                nc.sync.dma_start(
                    out=w,
                    in_=wdram[:, p * 128 : (p + 1) * 128].rearrange(
                        "(t p) o -> p t o", p=128
                    ),
                )
                wsl[which, p] = w

        # ---- V projection: 2 passes x 8 s-tiles over all 8 PSUM banks,
        # dt-outer so pass 1 consumes xt chunks as the DMA delivers them.
        def v_pass(sb):  # sb = base s-tile (0 or 8)
            t01 = psum.tile([128, 2, HL, DH], F32, name=f"psv{sb}a", tag="sp0", bufs=1)
            t23 = psum.tile([128, 2, HL, DH], F32, name=f"psv{sb}b", tag="sp1", bufs=1)
            singles_ = [
                psum.tile([128, HL, DH], F32, name=f"psv{sb}_{i}", tag=t, bufs=1)
                for i, t in enumerate(("c0", "c1", "proj", "tp"))
            ]
            dsts = [t01[:, 0], t01[:, 1], t23[:, 0], t23[:, 1]] + singles_
            for dt in range(8):
                for g in range(8):
                    st = sb + g
                    nc.tensor.matmul(
                        dsts[g],
                        xts[dt][:, st * 128 : (st + 1) * 128],
                        wv[:, dt, :],
                        start=(dt == 0),
                        stop=(dt == 7),
                    )
            for g in range(8):
                st = sb + g
                va = vaug[st]
                nc.vector.memset(va[:, :, DH : DH + 1], 1.0)
                nc.vector.tensor_add(va[:, :, 0:DH], dsts[g], bv_bc)

        v_pass(0)
        v_pass(8)

        # ---- QK projection machinery ----
        qk_tag = [0]

        def emit_qk_chunk(which, p, c, tags=("proj", "tp")):
            w = wsl[which, p]
            dst = {"k": kts, "q": qts}[which][p]
            bias_sb = {"k": bk_sb, "q": bq_sb}[which]
            tag = tags[qk_tag[0] % len(tags)]
            qk_tag[0] += 1
            ps = psum.tile([128, 512], F32, name=f"ps{which}{p}_{c}", tag=tag, bufs=1)
            for dt in range(8):
                nc.tensor.matmul(
                    ps,
                    w[:, dt, :],
                    xts[dt][:, c * 512 : (c + 1) * 512],
                    start=(dt == 0),
                    stop=(dt == 7),
                )
            nc.vector.tensor_scalar_add(
                dst[:, c * 512 : (c + 1) * 512], ps, bias_sb[:, p : p + 1]
            )

        # Upfront: all of kt(p0) (scores at (p0,qc0) span every key chunk)
        # plus qt(p0,c0).
        for c in range(4):
            emit_qk_chunk("k", 0, c)
        emit_qk_chunk("q", 0, 0)

        # Remaining chunks drip-fed into attention slots (one matmul at a
        # time) via this generator-like cursor.
        proj_chunks = [("q", 0, 1), ("q", 0, 2), ("q", 0, 3)]
        for p in range(1, NP):
            for c in range(4):
                proj_chunks.append(("k", p, c))
            for c in range(4):
                proj_chunks.append(("q", p, c))
        # flatten to per-matmul steps, pair-indexed deadlines
        proj_cursor = [0, 0]  # chunk index, dt index

        def proj_mms_left():
            ci, dt = proj_cursor
            return (len(proj_chunks) - ci) * 8 - dt

        def emit_proj_mm():
            ci, dt = proj_cursor
            if ci >= len(proj_chunks):
                return False
            which, p, c = proj_chunks[ci]
            if dt == 0:
                emit_proj_mm.ps = psum.tile(
                    [128, 512], F32, name=f"ps{which}{p}_{c}", tag="proj", bufs=1
                )
            w = wsl[which, p]
            nc.tensor.matmul(
                emit_proj_mm.ps,
                w[:, dt, :],
                xts[dt][:, c * 512 : (c + 1) * 512],
                start=(dt == 0),
                stop=(dt == 7),
            )
            if dt == 7:
                dst = {"k": kts, "q": qts}[which][p]
                bias_sb = {"k": bk_sb, "q": bq_sb}[which]
                nc.vector.tensor_scalar_add(
                    dst[:, c * 512 : (c + 1) * 512], emit_proj_mm.ps,
                    bias_sb[:, p : p + 1],
                )
                proj_cursor[0] += 1
                proj_cursor[1] = 0
            else:
                proj_cursor[1] += 1
            return True

        # ---- attention: global software pipeline over 256 (p,qc,jt) slots.
        # Slot g emits: scores(g) -> exp(g) -> [proj drip] -> ctx(g-2)
        # -> [one drain step].  Drains (from the previous qc) interleave
        # into slots 1..9 of the following qc.
        steps = [(p, qc, jt) for p in range(NP) for qc in range(QC) for jt in range(ST)]
        hist = {}  # g -> (u tile, p, qc, jt, C)
        Cs = {}  # (p, qc) -> [C0, C1]
        pending_drain = None  # dict with csb tiles, ot tile, p, qc, step counter
        drain_q = []

        def emit_scores(g, p, qc, jt):
            base = qc * 512
            sp = psum.tile(
                [128, 2, 512], F32, name=f"sp{g}", tag=f"sp{g % 2}", bufs=1
            )
            for x in range(2):
                hp = slice(x * 64, x * 64 + 64)
                nc.tensor.matmul(
                    sp[:, x, :],
                    kts[p][hp, jt * 128 : (jt + 1) * 128],
                    qts[p][hp, base : base + 512],
                    start=True,
                    stop=True,
                )
            u = attn.tile(
                [128, 2, 512], BF16, name=f"u{g}", tag=f"u{g % 3}", bufs=1
            )
            nc.scalar.activation(
                u, sp, EXP, bias=mask_sb[:, jt : jt + 1], scale=0.125
            )
            hist[g] = (u, p, qc, jt)

        def emit_ctx(g):
            u, p, qc, jt = hist.pop(g)
            if jt == 0:
                Cs[p, qc] = [
                    psum.tile(
                        [DH + 1, 512], F32, name=f"c{x}_{p}_{qc}", tag=f"c{x}", bufs=1
                    )
                    for x in range(2)
                ]
            C = Cs[p, qc]
            for x in range(2):
                nc.tensor.matmul(
                    C[x],
                    vaug[jt][:, 2 * p + x, :],
                    u[:, x, :],
                    start=(jt == 0),
                    stop=(jt == ST - 1),
                )
            if jt == ST - 1:
                start_drain(p, qc)

        def start_drain(p, qc):
            C = Cs.pop((p, qc))
            csb = []
            for x in range(2):
                cs = attn.tile(
                    [DH + 1, 512], F32, name=f"csb{p}_{qc}_{x}", tag=f"csb{x}", bufs=2
                )
                nc.vector.tensor_copy(out=cs, in_=C[x])
                csb.append(cs)
            ot = attn.tile([128, 4, 2, DH], F32, name=f"ot{p}_{qc}", tag="ot", bufs=2)
            drain_q.append({"csb": csb, "ot": ot, "p": p, "qc": qc, "i": 0})

        def emit_drain_step(final_tag=None):
            if not drain_q:
                return
            dr = drain_q[0]
            i = dr["i"]
            it, x = i // 2, i % 2
            tag = final_tag if final_tag else "tp"
            tp_ = psum.tile(
                [128, DH + 1], F32, name=f"tp{dr['p']}_{dr['qc']}_{i}", tag=tag, bufs=1
            )
            nc.tensor.transpose(
                tp_,
                dr["csb"][x][:, it * 128 : (it + 1) * 128],
                ident[0 : DH + 1, 0 : DH + 1],
            )
            rc = attn.tile(
                [128, 1], F32, name=f"rc{dr['p']}_{dr['qc']}_{i}", tag="rc", bufs=4
            )
            nc.vector.reciprocal(rc, tp_[:, DH : DH + 1])
            nc.vector.tensor_scalar_mul(dr["ot"][:, it, x, :], tp_[:, 0:DH], rc)
            dr["i"] += 1
            if dr["i"] == 8:
                p, qc = dr["p"], dr["qc"]
                base = qc * 512
                nc.sync.dma_start(
                    out=out[
                        base : base + 512, 2 * p * DH : (2 * p + 2) * DH
                    ].rearrange("(i p) c -> p i c", p=128),
                    in_=dr["ot"],
                )
                drain_q.pop(0)

        NSLOT = len(steps)
        for g, (p, qc, jt) in enumerate(steps):
            emit_scores(g, p, qc, jt)
            # proj drip: keep pair p+1's chunks on schedule within pair p.
            # Remaining slots in the current pair (including this one):
            pair_slots_left = 64 - (g % 64)
            # MMs that must finish by the end of this pair: everything
            # up to and including (q, p+1, 3) -- i.e. all chunks for
            # pairs <= p+1.  Approximation: emit evenly at
            # ceil(left/slots) capped at 2.
            left = proj_mms_left()
            if left > 0:
                quota = min(2, max(0, -(-left // max(1, (NSLOT - g)))))
                # ensure next pair's QK lands in time: required rate
                # within this pair
                ci = proj_cursor[0]
                need_this_pair = 0
                for k in range(ci, len(proj_chunks)):
                    wq_, pp, cc = proj_chunks[k]
                    if pp <= p + 1:
                        need_this_pair += 8
                    else:
                        break
                if need_this_pair:
                    need_this_pair -= proj_cursor[1]
                    quota = min(2, max(quota, -(-need_this_pair // pair_slots_left)))
                for _ in range(quota):
                    emit_proj_mm()
            if g >= 2:
                emit_ctx(g - 2)
            emit_drain_step()
        # pipeline tail
        emit_ctx(NSLOT - 2)
        emit_ctx(NSLOT - 1)
        k = 0
        while drain_q:
            emit_drain_step(final_tag=("proj", "tp")[k % 2])
            k += 1


def _make_in_maps(hidden_states, attention_mask, Wq, bq, Wk, bk, Wv, bv):
    bf = ml_dtypes.bfloat16
    in_maps = []
    for c in range(8):
        b, hg = divmod(c, 2)
        sl = slice(hg * O, (hg + 1) * O)
        in_maps.append(
            {
                "xt": np.ascontiguousarray(hidden_states[b].T.astype(bf)),
                "wqt": np.ascontiguousarray(Wq[sl, :].T.astype(bf)),
                "wkt": np.ascontiguousarray(Wk[sl, :].T.astype(bf)),
                "wvt": np.ascontiguousarray(Wv[sl, :].T.astype(bf)),
                "bq": np.ascontiguousarray(bq[sl]),
                "bk": np.ascontiguousarray(bk[sl]),
                "bv": np.ascontiguousarray(bv[sl]),
                "mask": np.ascontiguousarray(attention_mask[b, 0, 0, :]),
            }
        )
    return in_maps


def _gather(results):
    out = np.empty((B, S, D), dtype=np.float32)
    for c in range(8):
        b, hg = divmod(c, 2)
        out[b, :, hg * O : (hg + 1) * O] = results[c]["out"]
    return out


def kernel(hidden_states, attention_mask, Wq, bq, Wk, bk, Wv, bv, **run_kwargs):
    global _NC_CACHE
    args = [hidden_states, attention_mask, Wq, bq, Wk, bk, Wv, bv]
    args = [np.asarray(a, dtype=np.float32) for a in args]
    if _NC_CACHE is None:
        _NC_CACHE = build_nc()
    in_maps = _make_in_maps(*args)
    res = run_bass_kernel_spmd(_NC_CACHE, in_maps, core_ids=list(range(8)), **run_kwargs)
    kernel.last_result = res
    return _gather(res.results)


# revision 5
# speedup vs baseline: 1.2883x; 1.0425x over previous
"""BERT self-attention (B=4, S=2048, D=1024, H=16) on 8 trn2 NeuronCores.

Sharding: core c -> (batch b = c//2, head-group hg = c%2, 8 heads each).
Each core computes out[b, :, hg*512:(hg+1)*512] independently; host
gathers. Inputs are pre-transposed AND pre-tiled on host so every DMA is
contiguous >=4KB per partition: xt = X.T [D,S]; weights are partition-
major ([128, dt, ...] with row p holding W.T[dt*128+p, cols]).

v5 design (all-bf16, fully software-pipelined, ACT(exp)-paced):
  - V-projection raced against the xt DMA: dt-outer over ALL 8 PSUM
    banks (2 passes x 8 s-tiles); pass drains interleaved into the last
    dt row so DVE never serializes the pass handoff.
  - QK projection: kt(p0)+qt(p0,c0) up front; every remaining chunk
    drip-fed into attention jt-slots (PE slack while ACT runs exp),
    paced so pair p+1's Q/K finish during pair p, skipping the first 2
    slots of each qc (boundary pressure).
  - Attention per (pair, qc): scores for 2 heads as concurrent
    row-group matmuls -> [128,2,512] PSUM; ONE exp per jt ([128,1024]
    ACT op, mask as bias); ctx accumulates in PSUM and LAGS TWO SLOTS
    (sp bufs=2, u bufs=3) so PE never head-of-line blocks on exp and
    the C-bank drain gets slack before ctx(start=True) reuses it.
  - Drain: C -> SBUF copy (split per head, overlapped with last ctx),
    then 8 (transpose, reciprocal, scale) steps in slots 4..11 of the
    next qc; one batched DMA out per (p,qc).  Final drain fans out over
    4 free PSUM banks.
PSUM: sp0(2) + sp1(2) + c0(1) + c1(1) + proj(1) + tp(1) = 8 banks.
"""

import ml_dtypes
import numpy as np

import concourse.bass as bass
import concourse.tile as tile
from concourse import bacc, mybir
from concourse.bass_utils import run_bass_kernel_spmd
from concourse.masks import make_identity

B, S, D, H = 4, 2048, 1024, 16
DH = 64
O = 512  # per-core output width (8 heads)
HL = 8  # local heads per core
NP = 4  # head pairs per core
ST = S // 128  # 16 s-tiles
QC = 4  # query quarters (512 queries each)
F32 = mybir.dt.float32
BF16 = mybir.dt.bfloat16
EXP = mybir.ActivationFunctionType.Exp

_NC_CACHE = None


def build_nc():
    nc = bacc.Bacc(
        "TRN2",
        target_bir_lowering=False,
        debug=False,
        enable_asserts=True,
        num_devices=8,
    )
    xt = nc.dram_tensor("xt", [D, S], BF16, kind="ExternalInput").ap()
    # partition-major pre-tiled weights (see _make_in_maps)
    wqt = nc.dram_tensor("wqt", [128, NP, 8, 128], BF16, kind="ExternalInput").ap()
    wkt = nc.dram_tensor("wkt", [128, NP, 8, 128], BF16, kind="ExternalInput").ap()
    wvt = nc.dram_tensor("wvt", [128, 8, O], BF16, kind="ExternalInput").ap()
    bq = nc.dram_tensor("bq", [O], F32, kind="ExternalInput").ap()
    bk = nc.dram_tensor("bk", [O], F32, kind="ExternalInput").ap()
    bv = nc.dram_tensor("bv", [O], F32, kind="ExternalInput").ap()
    mask = nc.dram_tensor("mask", [S], F32, kind="ExternalInput").ap()
    out = nc.dram_tensor("out", [S, O], F32, kind="ExternalOutput").ap()

    with tile.TileContext(nc) as tc:
        _emit(nc, tc, xt, wqt, wkt, wvt, bq, bk, bv, mask, out)
    nc.compile()
    return nc


def _emit(nc, tc, xt, wqt, wkt, wvt, bq, bk, bv, mask, out):
    with (
        tc.tile_pool(name="singles", bufs=1) as singles,
        tc.tile_pool(name="persist", bufs=1) as persist,
        tc.tile_pool(name="wpool", bufs=1) as wpool,
        tc.tile_pool(name="attn", bufs=1) as attn,
        tc.tile_pool(name="psum", bufs=1, space="PSUM") as psum,
    ):
        # persistent activations (all bf16)
        xts = [persist.tile([128, S], BF16, name=f"xts{dt}", tag=f"xts{dt}") for dt in range(8)]
        qts = [persist.tile([128, S], BF16, name=f"qt{p}", tag=f"qt{p}") for p in range(NP)]
        kts = [persist.tile([128, S], BF16, name=f"kt{p}", tag=f"kt{p}") for p in range(NP)]
        vaug = [
            persist.tile([128, HL, DH + 1], BF16, name=f"vaug{t}", tag=f"vaug{t}")
            for t in range(ST)
        ]

        # DMA order is the startup critical path: wv first (V proj races
        # the xt stream), then xt, then small/late-needed tensors, then
        # QK weights (first used ~35us in).
        wv = wpool.tile([128, 8, O], BF16, name="wv", tag="wv")
        nc.sync.dma_start(out=wv[:, 0:4, :], in_=wvt[:, 0:4, :])
        nc.sync.dma_start(out=xts[0], in_=xt[0:128, :])
        nc.sync.dma_start(out=xts[1], in_=xt[128:256, :])
        nc.sync.dma_start(out=wv[:, 4:8, :], in_=wvt[:, 4:8, :])
        for dt in range(2, 8):
            nc.sync.dma_start(out=xts[dt], in_=xt[dt * 128 : (dt + 1) * 128, :])
        ident = singles.tile([128, 128], F32)
        make_identity(nc, ident)
        mask_sb = singles.tile([128, ST], F32)
        nc.sync.dma_start(out=mask_sb, in_=mask.rearrange("(t p) -> p t", p=128))
        bq_sb = singles.tile([128, NP], F32)
        nc.sync.dma_start(out=bq_sb, in_=bq.rearrange("(t p) -> p t", p=128))
        bk_sb = singles.tile([128, NP], F32)
        nc.sync.dma_start(out=bk_sb, in_=bk.rearrange("(t p) -> p t", p=128))
        bv_bc = singles.tile([128, HL, DH], F32)
        nc.sync.dma_start(
            out=bv_bc, in_=bass.AP(tensor=bv.tensor, offset=0, ap=[[0, 128], [1, O]])
        )
        wk = wpool.tile([128, NP, 8, 128], BF16, name="wk", tag="wk")
        nc.sync.dma_start(out=wk, in_=wkt)
        wq = wpool.tile([128, NP, 8, 128], BF16, name="wq", tag="wq")
        nc.sync.dma_start(out=wq, in_=wqt)
        wsl = {"k": wk, "q": wq}

        # vaug ones-columns: DVE is idle now, do them all up front
        for st in range(ST):
            nc.vector.memset(vaug[st][:, :, DH : DH + 1], 1.0)

        # ---- V projection: 2 passes x 8 s-tiles over all 8 PSUM banks,
        # dt-outer so pass 1 consumes xt chunks as the DMA delivers them.
        # Drains interleave into the dt=7 row so the pass handoff never
        # serializes on DVE.
        def v_pass(sb):  # sb = base s-tile (0 or 8)
            t01 = psum.tile([128, 2, HL, DH], F32, name=f"psv{sb}a", tag="sp0", bufs=1)
            t23 = psum.tile([128, 2, HL, DH], F32, name=f"psv{sb}b", tag="sp1", bufs=1)
            singles_ = [
                psum.tile([128, HL, DH], F32, name=f"psv{sb}_{i}", tag=t, bufs=1)
                for i, t in enumerate(("c0", "c1", "proj", "tp"))
            ]
            dsts = [t01[:, 0], t01[:, 1], t23[:, 0], t23[:, 1]] + singles_
            for dt in range(8):
                for g in range(8):
                    st = sb + g
                    nc.tensor.matmul(
                        dsts[g],
                        xts[dt][:, st * 128 : (st + 1) * 128],
                        wv[:, dt, :],
                        start=(dt == 0),
                        stop=(dt == 7),
                    )
                    if dt == 7:
                        va = vaug[st]
                        nc.vector.tensor_add(va[:, :, 0:DH], dsts[g], bv_bc)

        v_pass(0)
        v_pass(8)

        # ---- QK projection machinery ----
        qk_tag = [0]

        def emit_qk_chunk(which, p, c):
            w = wsl[which]
            dst = {"k": kts, "q": qts}[which][p]
            bias_sb = {"k": bk_sb, "q": bq_sb}[which]
            tag = ("proj", "tp")[qk_tag[0] % 2]
            qk_tag[0] += 1
            ps = psum.tile([128, 512], F32, name=f"ps{which}{p}_{c}", tag=tag, bufs=1)
            for dt in range(8):
                nc.tensor.matmul(
                    ps,
                    w[:, p, dt, :],
                    xts[dt][:, c * 512 : (c + 1) * 512],
                    start=(dt == 0),
                    stop=(dt == 7),
                )
            nc.vector.tensor_scalar_add(
                dst[:, c * 512 : (c + 1) * 512], ps, bias_sb[:, p : p + 1]
            )

        # Upfront: all of kt(p0) (scores at (p0,qc0) span every key chunk)
        # plus qt(p0,c0).
        for c in range(4):
            emit_qk_chunk("k", 0, c)
        emit_qk_chunk("q", 0, 0)

        # Remaining chunks drip-fed into attention slots.
        proj_chunks = [("q", 0, 1), ("q", 0, 2), ("q", 0, 3)]
        for p in range(1, NP):
            for c in range(4):
                proj_chunks.append(("k", p, c))
            for c in range(4):
                proj_chunks.append(("q", p, c))
        proj_cursor = [0, 0]  # chunk index, dt index

        def proj_mms_left():
            ci, dt = proj_cursor
            return (len(proj_chunks) - ci) * 8 - dt

        def emit_proj_mm():
            ci, dt = proj_cursor
            if ci >= len(proj_chunks):
                return False
            which, p, c = proj_chunks[ci]
            if dt == 0:
                emit_proj_mm.ps = psum.tile(
                    [128, 512], F32, name=f"ps{which}{p}_{c}", tag="proj", bufs=1
                )
            nc.tensor.matmul(
                emit_proj_mm.ps,
                wsl[which][:, p, dt, :],
                xts[dt][:, c * 512 : (c + 1) * 512],
                start=(dt == 0),
                stop=(dt == 7),
            )
            if dt == 7:
                dst = {"k": kts, "q": qts}[which][p]
                bias_sb = {"k": bk_sb, "q": bq_sb}[which]
                nc.vector.tensor_scalar_add(
                    dst[:, c * 512 : (c + 1) * 512], emit_proj_mm.ps,
                    bias_sb[:, p : p + 1],
                )
                proj_cursor[0] += 1
                proj_cursor[1] = 0
            else:
                proj_cursor[1] += 1
            return True

        # ---- attention: global software pipeline over 256 (p,qc,jt)
        # slots.  Slot g: scores(g) -> exp(g) -> [proj drip] -> ctx(g-2)
        # -> [drain step in slots 4..11].
        steps = [(p, qc, jt) for p in range(NP) for qc in range(QC) for jt in range(ST)]
        hist = {}
        Cs = {}
        drain_q = []

        def emit_scores(g, p, qc, jt):
            base = qc * 512
            sp = psum.tile([128, 2, 512], F32, name=f"sp{g}", tag=f"sp{g % 2}", bufs=1)
            for x in range(2):
                hp = slice(x * 64, x * 64 + 64)
                nc.tensor.matmul(
                    sp[:, x, :],
                    kts[p][hp, jt * 128 : (jt + 1) * 128],
                    qts[p][hp, base : base + 512],
                    start=True,
                    stop=True,
                )
            u = attn.tile([128, 2, 512], BF16, name=f"u{g}", tag=f"u{g % 3}", bufs=1)
            nc.scalar.activation(u, sp, EXP, bias=mask_sb[:, jt : jt + 1], scale=0.125)
            hist[g] = (u, p, qc, jt)

        def emit_ctx(g):
            u, p, qc, jt = hist.pop(g)
            if jt == 0:
                Cs[p, qc] = [
                    psum.tile(
                        [DH + 1, 512], F32, name=f"c{x}_{p}_{qc}", tag=f"c{x}", bufs=1
                    )
                    for x in range(2)
                ]
            C = Cs[p, qc]
            last = jt == ST - 1
            csb = []
            for x in range(2):
                nc.tensor.matmul(
                    C[x],
                    vaug[jt][:, 2 * p + x, :],
                    u[:, x, :],
                    start=(jt == 0),
                    stop=last,
                )
                if last:
                    cs = attn.tile(
                        [DH + 1, 512], F32, name=f"csb{p}_{qc}_{x}", tag=f"csb{x}",
                        bufs=2,
                    )
                    nc.vector.tensor_copy(out=cs, in_=C[x])
                    csb.append(cs)
            if last:
                Cs.pop((p, qc))
                ot = attn.tile(
                    [128, 4, 2, DH], F32, name=f"ot{p}_{qc}", tag="ot", bufs=2
                )
                drain_q.append({"csb": csb, "ot": ot, "p": p, "qc": qc, "i": 0})

        def emit_drain_step(tag="tp"):
            if not drain_q:
                return
            dr = drain_q[0]
            i = dr["i"]
            it, x = i // 2, i % 2
            tp_ = psum.tile(
                [128, DH + 1], F32, name=f"tp{dr['p']}_{dr['qc']}_{i}", tag=tag, bufs=1
            )
            nc.tensor.transpose(
                tp_,
                dr["csb"][x][:, it * 128 : (it + 1) * 128],
                ident[0 : DH + 1, 0 : DH + 1],
            )
            rc = attn.tile(
                [128, 1], F32, name=f"rc{dr['p']}_{dr['qc']}_{i}", tag="rc", bufs=4
            )
            nc.vector.reciprocal(rc, tp_[:, DH : DH + 1])
            nc.vector.tensor_scalar_mul(dr["ot"][:, it, x, :], tp_[:, 0:DH], rc)
            dr["i"] += 1
            if dr["i"] == 8:
                p, qc = dr["p"], dr["qc"]
                base = qc * 512
                nc.sync.dma_start(
                    out=out[
                        base : base + 512, 2 * p * DH : (2 * p + 2) * DH
                    ].rearrange("(i p) c -> p i c", p=128),
                    in_=dr["ot"],
                )
                drain_q.pop(0)

        NSLOT = len(steps)
        for g, (p, qc, jt) in enumerate(steps):
            emit_scores(g, p, qc, jt)
            left = proj_mms_left()
            if left > 0 and jt >= 2:
                quota = min(2, max(0, -(-left // max(1, (NSLOT - g)))))
                ci = proj_cursor[0]
                need_this_pair = 0
                for k in range(ci, len(proj_chunks)):
                    if proj_chunks[k][1] <= p + 1:
                        need_this_pair += 8
                    else:
                        break
                if need_this_pair:
                    need_this_pair -= proj_cursor[1]
                    pair_slots_left = 64 - (g % 64)
                    quota = min(2, max(quota, -(-need_this_pair // pair_slots_left)))
                for _ in range(quota):
                    emit_proj_mm()
            if g >= 2:
                emit_ctx(g - 2)
            if 4 <= jt <= 11:
                emit_drain_step()
        # pipeline tail: last two ctx slots, then the final drain fanned
        # out over the 4 now-free PSUM banks.
        emit_ctx(NSLOT - 2)
        emit_ctx(NSLOT - 1)
        k = 0
        while drain_q:
            emit_drain_step(tag=("proj", "tp", "c0", "c1")[k % 4])
            k += 1


def _make_in_maps(hidden_states, attention_mask, Wq, bq, Wk, bk, Wv, bv):
    bf = ml_dtypes.bfloat16

    def wqk_tiled(W, sl):
        # W.T shard [D, O] -> [128, NP, 8, 128]: row p holds
        # W.T[dt*128+p, pair*128:(pair+1)*128] at [p, pair, dt, :]
        wt = W[sl, :].T.astype(bf)  # [D, O]
        return np.ascontiguousarray(
            wt.reshape(8, 128, NP, 128).transpose(1, 2, 0, 3)
        )

    def wv_tiled(W, sl):
        wt = W[sl, :].T.astype(bf)  # [D, O]
        return np.ascontiguousarray(wt.reshape(8, 128, O).transpose(1, 0, 2))

    in_maps = []
    for c in range(8):
        b, hg = divmod(c, 2)
        sl = slice(hg * O, (hg + 1) * O)
        in_maps.append(
            {
                "xt": np.ascontiguousarray(hidden_states[b].T.astype(bf)),
                "wqt": wqk_tiled(Wq, sl),
                "wkt": wqk_tiled(Wk, sl),
                "wvt": wv_tiled(Wv, sl),
                "bq": np.ascontiguousarray(bq[sl]),
                "bk": np.ascontiguousarray(bk[sl]),
                "bv": np.ascontiguousarray(bv[sl]),
                "mask": np.ascontiguousarray(attention_mask[b, 0, 0, :]),
            }
        )
    return in_maps


def _gather(results):
    out = np.empty((B, S, D), dtype=np.float32)
    for c in range(8):
        b, hg = divmod(c, 2)
        out[b, :, hg * O : (hg + 1) * O] = results[c]["out"]
    return out


def kernel(hidden_states, attention_mask, Wq, bq, Wk, bk, Wv, bv, **run_kwargs):
    global _NC_CACHE
    args = [hidden_states, attention_mask, Wq, bq, Wk, bk, Wv, bv]
    args = [np.asarray(a, dtype=np.float32) for a in args]
    if _NC_CACHE is None:
        _NC_CACHE = build_nc()
    in_maps = _make_in_maps(*args)
    res = run_bass_kernel_spmd(_NC_CACHE, in_maps, core_ids=list(range(8)), **run_kwargs)
    kernel.last_result = res
    return _gather(res.results)


# revision 6
# speedup vs baseline: 1.2983x; 1.0077x over previous
"""BERT self-attention (B=4, S=2048, D=1024, H=16) on 8 trn2 NeuronCores.

Sharding: core c -> (batch b = c//2, head-group hg = c%2, 8 heads each).
Each core computes out[b, :, hg*512:(hg+1)*512] independently; host
gathers. Inputs are pre-transposed AND pre-tiled on host so every DMA is
contiguous >=4KB per partition: xt = X.T [D,S]; weights are partition-
major ([128, dt, ...] with row p holding W.T[dt*128+p, cols]).

v5 design (all-bf16, fully software-pipelined, ACT(exp)-paced):
  - V-projection raced against the xt DMA: dt-outer over ALL 8 PSUM
    banks (2 passes x 8 s-tiles); pass drains interleaved into the last
    dt row so DVE never serializes the pass handoff.
  - QK projection: kt(p0)+qt(p0,c0) up front; every remaining chunk
    drip-fed into attention jt-slots (PE slack while ACT runs exp),
    paced so pair p+1's Q/K finish during pair p, skipping the first 2
    slots of each qc (boundary pressure).
  - Attention per (pair, qc): scores for 2 heads as concurrent
    row-group matmuls -> [128,2,512] PSUM; ONE exp per jt ([128,1024]
    ACT op, mask as bias); ctx accumulates in PSUM and LAGS TWO SLOTS
    (sp bufs=2, u bufs=3) so PE never head-of-line blocks on exp and
    the C-bank drain gets slack before ctx(start=True) reuses it.
  - Drain: C -> SBUF copy (split per head, overlapped with last ctx),
    then 8 (transpose, reciprocal, scale) steps in slots 4..11 of the
    next qc; one batched DMA out per (p,qc).  Final drain fans out over
    4 free PSUM banks.
PSUM: sp0(2) + sp1(2) + c0(1) + c1(1) + proj(1) + tp(1) = 8 banks.
"""

import ml_dtypes
import numpy as np

import concourse.bass as bass
import concourse.tile as tile
from concourse import bacc, mybir
from concourse.bass_utils import run_bass_kernel_spmd
from concourse.masks import make_identity

B, S, D, H = 4, 2048, 1024, 16
DH = 64
O = 512  # per-core output width (8 heads)
HL = 8  # local heads per core
NP = 4  # head pairs per core
ST = S // 128  # 16 s-tiles
QC = 4  # query quarters (512 queries each)
F32 = mybir.dt.float32
BF16 = mybir.dt.bfloat16
EXP = mybir.ActivationFunctionType.Exp

_NC_CACHE = None


def build_nc():
    nc = bacc.Bacc(
        "TRN2",
        target_bir_lowering=False,
        debug=False,
        enable_asserts=True,
        num_devices=8,
    )
    xt = nc.dram_tensor("xt", [D, S], BF16, kind="ExternalInput").ap()
    # partition-major pre-tiled weights (see _make_in_maps)
    wqt = nc.dram_tensor("wqt", [128, NP, 8, 128], BF16, kind="ExternalInput").ap()
    wkt = nc.dram_tensor("wkt", [128, NP, 8, 128], BF16, kind="ExternalInput").ap()
    wvt = nc.dram_tensor("wvt", [128, 8, O], BF16, kind="ExternalInput").ap()
    bq = nc.dram_tensor("bq", [O], F32, kind="ExternalInput").ap()
    bk = nc.dram_tensor("bk", [O], F32, kind="ExternalInput").ap()
    bv = nc.dram_tensor("bv", [O], F32, kind="ExternalInput").ap()
    mask = nc.dram_tensor("mask", [S], F32, kind="ExternalInput").ap()
    out = nc.dram_tensor("out", [S, O], F32, kind="ExternalOutput").ap()

    with tile.TileContext(nc) as tc:
        _emit(nc, tc, xt, wqt, wkt, wvt, bq, bk, bv, mask, out)
    nc.compile()
    return nc


def _emit(nc, tc, xt, wqt, wkt, wvt, bq, bk, bv, mask, out):
    with (
        tc.tile_pool(name="singles", bufs=1) as singles,
        tc.tile_pool(name="persist", bufs=1) as persist,
        tc.tile_pool(name="wpool", bufs=1) as wpool,
        tc.tile_pool(name="attn", bufs=1) as attn,
        tc.tile_pool(name="psum", bufs=1, space="PSUM") as psum,
    ):
        # persistent activations (all bf16)
        xts = [persist.tile([128, S], BF16, name=f"xts{dt}", tag=f"xts{dt}") for dt in range(8)]
        qts = [persist.tile([128, S], BF16, name=f"qt{p}", tag=f"qt{p}") for p in range(NP)]
        kts = [persist.tile([128, S], BF16, name=f"kt{p}", tag=f"kt{p}") for p in range(NP)]
        vaug = [
            persist.tile([128, HL, DH + 1], BF16, name=f"vaug{t}", tag=f"vaug{t}")
            for t in range(ST)
        ]

        # DMA order is the startup critical path: wv first (V proj races
        # the xt stream), then xt, then small/late-needed tensors, then
        # QK weights (first used ~35us in).
        wv = wpool.tile([128, 8, O], BF16, name="wv", tag="wv")
        nc.sync.dma_start(out=wv[:, 0:1, :], in_=wvt[:, 0:1, :])
        nc.sync.dma_start(out=xts[0], in_=xt[0:128, :])
        nc.sync.dma_start(out=wv[:, 1:4, :], in_=wvt[:, 1:4, :])
        bv_bc = singles.tile([128, HL, DH], F32)
        nc.sync.dma_start(
            out=bv_bc, in_=bass.AP(tensor=bv.tensor, offset=0, ap=[[0, 128], [1, O]])
        )
        nc.sync.dma_start(out=xts[1], in_=xt[128:256, :])
        nc.sync.dma_start(out=wv[:, 4:8, :], in_=wvt[:, 4:8, :])
        for dt in range(2, 8):
            nc.sync.dma_start(out=xts[dt], in_=xt[dt * 128 : (dt + 1) * 128, :])
        ident = singles.tile([128, 128], F32)
        make_identity(nc, ident)
        wk = wpool.tile([128, NP, 8, 128], BF16, name="wk", tag="wk")
        nc.sync.dma_start(out=wk, in_=wkt)
        wq = wpool.tile([128, NP, 8, 128], BF16, name="wq", tag="wq")
        nc.sync.dma_start(out=wq, in_=wqt)
        wsl = {"k": wk, "q": wq}
        bq_sb = singles.tile([128, NP], F32)
        nc.sync.dma_start(out=bq_sb, in_=bq.rearrange("(t p) -> p t", p=128))
        bk_sb = singles.tile([128, NP], F32)
        nc.sync.dma_start(out=bk_sb, in_=bk.rearrange("(t p) -> p t", p=128))
        mask_sb = singles.tile([128, ST], F32)
        nc.sync.dma_start(out=mask_sb, in_=mask.rearrange("(t p) -> p t", p=128))

        # vaug ones-columns: DVE is idle now, do them all up front
        for st in range(ST):
            nc.vector.memset(vaug[st][:, :, DH : DH + 1], 1.0)

        # ---- V projection: 2 passes x 8 s-tiles over all 8 PSUM banks,
        # dt-outer so pass 1 consumes xt chunks as the DMA delivers them.
        # Drains interleave into the dt=7 row so the pass handoff never
        # serializes on DVE.
        def v_pass(sb):  # sb = base s-tile (0 or 8)
            t01 = psum.tile([128, 2, HL, DH], F32, name=f"psv{sb}a", tag="sp0", bufs=1)
            t23 = psum.tile([128, 2, HL, DH], F32, name=f"psv{sb}b", tag="sp1", bufs=1)
            singles_ = [
                psum.tile([128, HL, DH], F32, name=f"psv{sb}_{i}", tag=t, bufs=1)
                for i, t in enumerate(("c0", "c1", "proj", "tp"))
            ]
            dsts = [t01[:, 0], t01[:, 1], t23[:, 0], t23[:, 1]] + singles_
            for dt in range(8):
                for g in range(8):
                    st = sb + g
                    nc.tensor.matmul(
                        dsts[g],
                        xts[dt][:, st * 128 : (st + 1) * 128],
                        wv[:, dt, :],
                        start=(dt == 0),
                        stop=(dt == 7),
                    )
                    if dt == 7:
                        va = vaug[st]
                        nc.vector.tensor_add(va[:, :, 0:DH], dsts[g], bv_bc)

        v_pass(0)
        v_pass(8)

        # ---- QK projection machinery ----
        qk_tag = [0]

        def emit_qk_chunk(which, p, c):
            w = wsl[which]
            dst = {"k": kts, "q": qts}[which][p]
            bias_sb = {"k": bk_sb, "q": bq_sb}[which]
            tag = ("proj", "tp")[qk_tag[0] % 2]
            qk_tag[0] += 1
            ps = psum.tile([128, 512], F32, name=f"ps{which}{p}_{c}", tag=tag, bufs=1)
            for dt in range(8):
                nc.tensor.matmul(
                    ps,
                    w[:, p, dt, :],
                    xts[dt][:, c * 512 : (c + 1) * 512],
                    start=(dt == 0),
                    stop=(dt == 7),
                )
            nc.vector.tensor_scalar_add(
                dst[:, c * 512 : (c + 1) * 512], ps, bias_sb[:, p : p + 1]
            )

        # Upfront: all of kt(p0) (scores at (p0,qc0) span every key chunk)
        # plus qt(p0,c0).
        for c in range(4):
            emit_qk_chunk("k", 0, c)
        emit_qk_chunk("q", 0, 0)

        # Remaining chunks drip-fed into attention slots.
        proj_chunks = [("q", 0, 1), ("q", 0, 2), ("q", 0, 3)]
        for p in range(1, NP):
            for c in range(4):
                proj_chunks.append(("k", p, c))
            for c in range(4):
                proj_chunks.append(("q", p, c))
        proj_cursor = [0, 0]  # chunk index, dt index

        def proj_mms_left():
            ci, dt = proj_cursor
            return (len(proj_chunks) - ci) * 8 - dt

        def emit_proj_mm():
            ci, dt = proj_cursor
            if ci >= len(proj_chunks):
                return False
            which, p, c = proj_chunks[ci]
            if dt == 0:
                emit_proj_mm.ps = psum.tile(
                    [128, 512], F32, name=f"ps{which}{p}_{c}", tag="proj", bufs=1
                )
            nc.tensor.matmul(
                emit_proj_mm.ps,
                wsl[which][:, p, dt, :],
                xts[dt][:, c * 512 : (c + 1) * 512],
                start=(dt == 0),
                stop=(dt == 7),
            )
            if dt == 7:
                dst = {"k": kts, "q": qts}[which][p]
                bias_sb = {"k": bk_sb, "q": bq_sb}[which]
                nc.vector.tensor_scalar_add(
                    dst[:, c * 512 : (c + 1) * 512], emit_proj_mm.ps,
                    bias_sb[:, p : p + 1],
                )
                proj_cursor[0] += 1
                proj_cursor[1] = 0
            else:
                proj_cursor[1] += 1
            return True

        # ---- attention: global software pipeline over 256 (p,qc,jt)
        # slots.  Slot g: scores(g) -> exp(g) -> [proj drip] -> ctx(g-2)
        # -> [drain step in slots 4..11].
        steps = [(p, qc, jt) for p in range(NP) for qc in range(QC) for jt in range(ST)]
        hist = {}
        Cs = {}
        drain_q = []

        def emit_scores(g, p, qc, jt):
            base = qc * 512
            sp = psum.tile([128, 2, 512], F32, name=f"sp{g}", tag=f"sp{g % 2}", bufs=1)
            for x in range(2):
                hp = slice(x * 64, x * 64 + 64)
                nc.tensor.matmul(
                    sp[:, x, :],
                    kts[p][hp, jt * 128 : (jt + 1) * 128],
                    qts[p][hp, base : base + 512],
                    start=True,
                    stop=True,
                )
            u = attn.tile([128, 2, 512], BF16, name=f"u{g}", tag=f"u{g % 3}", bufs=1)
            nc.scalar.activation(u, sp, EXP, bias=mask_sb[:, jt : jt + 1], scale=0.125)
            hist[g] = (u, p, qc, jt)

        def emit_ctx(g):
            u, p, qc, jt = hist.pop(g)
            if jt == 0:
                Cs[p, qc] = [
                    psum.tile(
                        [DH + 1, 512], F32, name=f"c{x}_{p}_{qc}", tag=f"c{x}", bufs=1
                    )
                    for x in range(2)
                ]
            C = Cs[p, qc]
            last = jt == ST - 1
            csb = []
            for x in range(2):
                nc.tensor.matmul(
                    C[x],
                    vaug[jt][:, 2 * p + x, :],
                    u[:, x, :],
                    start=(jt == 0),
                    stop=last,
                )
                if last:
                    cs = attn.tile(
                        [DH + 1, 512], F32, name=f"csb{p}_{qc}_{x}", tag=f"csb{x}",
                        bufs=2,
                    )
                    nc.vector.tensor_copy(out=cs, in_=C[x])
                    csb.append(cs)
            if last:
                Cs.pop((p, qc))
                ot = attn.tile(
                    [128, 4, 2, DH], F32, name=f"ot{p}_{qc}", tag="ot", bufs=2
                )
                drain_q.append({"csb": csb, "ot": ot, "p": p, "qc": qc, "i": 0})

        def emit_drain_step(tag="tp"):
            if not drain_q:
                return
            dr = drain_q[0]
            i = dr["i"]
            it, x = i // 2, i % 2
            tp_ = psum.tile(
                [128, DH + 1], F32, name=f"tp{dr['p']}_{dr['qc']}_{i}", tag=tag, bufs=1
            )
            nc.tensor.transpose(
                tp_,
                dr["csb"][x][:, it * 128 : (it + 1) * 128],
                ident[0 : DH + 1, 0 : DH + 1],
            )
            rc = attn.tile(
                [128, 1], F32, name=f"rc{dr['p']}_{dr['qc']}_{i}", tag="rc", bufs=4
            )
            nc.vector.reciprocal(rc, tp_[:, DH : DH + 1])
            nc.vector.tensor_scalar_mul(dr["ot"][:, it, x, :], tp_[:, 0:DH], rc)
            dr["i"] += 1
            if dr["i"] == 8:
                p, qc = dr["p"], dr["qc"]
                base = qc * 512
                nc.sync.dma_start(
                    out=out[
                        base : base + 512, 2 * p * DH : (2 * p + 2) * DH
                    ].rearrange("(i p) c -> p i c", p=128),
                    in_=dr["ot"],
                )
                drain_q.pop(0)

        NSLOT = len(steps)
        for g, (p, qc, jt) in enumerate(steps):
            emit_scores(g, p, qc, jt)
            left = proj_mms_left()
            if left > 0 and jt >= 2:
                quota = min(2, max(0, -(-left // max(1, (NSLOT - g)))))
                ci = proj_cursor[0]
                need_this_pair = 0
                for k in range(ci, len(proj_chunks)):
                    if proj_chunks[k][1] <= p + 1:
                        need_this_pair += 8
                    else:
                        break
                if need_this_pair:
                    need_this_pair -= proj_cursor[1]
                    pair_slots_left = 64 - (g % 64)
                    quota = min(2, max(quota, -(-need_this_pair // pair_slots_left)))
                for _ in range(quota):
                    emit_proj_mm()
            if g >= 2:
                emit_ctx(g - 2)
            if 4 <= jt <= 11:
                emit_drain_step()
        # pipeline tail: last two ctx slots, then the final drain fanned
        # out over the 4 now-free PSUM banks.
        emit_ctx(NSLOT - 2)
        emit_ctx(NSLOT - 1)
        k = 0
        while drain_q:
            emit_drain_step(tag=("proj", "tp", "c0", "c1")[k % 4])
            k += 1


def _make_in_maps(hidden_states, attention_mask, Wq, bq, Wk, bk, Wv, bv):
    bf = ml_dtypes.bfloat16

    def wqk_tiled(W, sl):
        # W.T shard [D, O] -> [128, NP, 8, 128]: row p holds
        # W.T[dt*128+p, pair*128:(pair+1)*128] at [p, pair, dt, :]
        wt = W[sl, :].T.astype(bf)  # [D, O]
        return np.ascontiguousarray(
            wt.reshape(8, 128, NP, 128).transpose(1, 2, 0, 3)
        )

    def wv_tiled(W, sl):
        wt = W[sl, :].T.astype(bf)  # [D, O]
        return np.ascontiguousarray(wt.reshape(8, 128, O).transpose(1, 0, 2))

    in_maps = []
    for c in range(8):
        b, hg = divmod(c, 2)
        sl = slice(hg * O, (hg + 1) * O)
        in_maps.append(
            {
                "xt": np.ascontiguousarray(hidden_states[b].T.astype(bf)),
                "wqt": wqk_tiled(Wq, sl),
                "wkt": wqk_tiled(Wk, sl),
                "wvt": wv_tiled(Wv, sl),
                "bq": np.ascontiguousarray(bq[sl]),
                "bk": np.ascontiguousarray(bk[sl]),
                "bv": np.ascontiguousarray(bv[sl]),
                "mask": np.ascontiguousarray(attention_mask[b, 0, 0, :]),
            }
        )
    return in_maps


def _gather(results):
    out = np.empty((B, S, D), dtype=np.float32)
    for c in range(8):
        b, hg = divmod(c, 2)
        out[b, :, hg * O : (hg + 1) * O] = results[c]["out"]
    return out


def kernel(hidden_states, attention_mask, Wq, bq, Wk, bk, Wv, bv, **run_kwargs):
    global _NC_CACHE
    args = [hidden_states, attention_mask, Wq, bq, Wk, bk, Wv, bv]
    args = [np.asarray(a, dtype=np.float32) for a in args]
    if _NC_CACHE is None:
        _NC_CACHE = build_nc()
    in_maps = _make_in_maps(*args)
    res = run_bass_kernel_spmd(_NC_CACHE, in_maps, core_ids=list(range(8)), **run_kwargs)
    kernel.last_result = res
    return _gather(res.results)


# revision 9
# speedup vs baseline: 1.3018x; 1.0027x over previous
"""BERT self-attention (B=4, S=2048, D=1024, H=16) on 8 trn2 NeuronCores.

Sharding: core c -> (batch b = c//2, head-group hg = c%2, 8 heads each).
Each core computes out[b, :, hg*512:(hg+1)*512] independently; host
gathers. Inputs are pre-transposed AND pre-tiled on host so every DMA is
contiguous >=4KB per partition: xt = X.T [D,S]; weights are partition-
major ([128, dt, ...] with row p holding W.T[dt*128+p, cols]).

v5 design (all-bf16, fully software-pipelined, ACT(exp)-paced):
  - V-projection raced against the xt DMA: dt-outer over ALL 8 PSUM
    banks (2 passes x 8 s-tiles); pass drains interleaved into the last
    dt row so DVE never serializes the pass handoff.
  - QK projection: kt(p0)+qt(p0,c0) up front; every remaining chunk
    drip-fed into attention jt-slots (PE slack while ACT runs exp),
    paced so pair p+1's Q/K finish during pair p, skipping the first 2
    slots of each qc (boundary pressure).
  - Attention per (pair, qc): scores for 2 heads as concurrent
    row-group matmuls -> [128,2,512] PSUM; ONE exp per jt ([128,1024]
    ACT op, mask as bias); ctx accumulates in PSUM and LAGS TWO SLOTS
    (sp bufs=2, u bufs=3) so PE never head-of-line blocks on exp and
    the C-bank drain gets slack before ctx(start=True) reuses it.
  - Drain: C -> SBUF copy (split per head, overlapped with last ctx),
    then 8 (transpose, reciprocal, scale) steps in slots 4..11 of the
    next qc; one batched DMA out per (p,qc).  Final drain fans out over
    4 free PSUM banks.
PSUM: sp0(2) + sp1(2) + c0(1) + c1(1) + proj(1) + tp(1) = 8 banks.
"""

import ml_dtypes
import numpy as np

import concourse.bass as bass
import concourse.tile as tile
from concourse import bacc, mybir
from concourse.bass_utils import run_bass_kernel_spmd
from concourse.masks import make_identity

B, S, D, H = 4, 2048, 1024, 16
DH = 64
O = 512  # per-core output width (8 heads)
HL = 8  # local heads per core
NP = 4  # head pairs per core
ST = S // 128  # 16 s-tiles
QC = 4  # query quarters (512 queries each)
F32 = mybir.dt.float32
BF16 = mybir.dt.bfloat16
EXP = mybir.ActivationFunctionType.Exp

_NC_CACHE = None


def build_nc():
    nc = bacc.Bacc(
        "TRN2",
        target_bir_lowering=False,
        debug=False,
        enable_asserts=True,
        num_devices=8,
    )
    xt = nc.dram_tensor("xt", [D, S], BF16, kind="ExternalInput").ap()
    # partition-major pre-tiled weights (see _make_in_maps)
    wqt = nc.dram_tensor("wqt", [128, NP, 8, 128], BF16, kind="ExternalInput").ap()
    wkt = nc.dram_tensor("wkt", [128, NP, 8, 128], BF16, kind="ExternalInput").ap()
    wvt = nc.dram_tensor("wvt", [128, 8, O], BF16, kind="ExternalInput").ap()
    bq = nc.dram_tensor("bq", [O], F32, kind="ExternalInput").ap()
    bk = nc.dram_tensor("bk", [O], F32, kind="ExternalInput").ap()
    bv = nc.dram_tensor("bv", [O], F32, kind="ExternalInput").ap()
    mask = nc.dram_tensor("mask", [S], F32, kind="ExternalInput").ap()
    out = nc.dram_tensor("out", [S, O], F32, kind="ExternalOutput").ap()

    with tile.TileContext(nc) as tc:
        _emit(nc, tc, xt, wqt, wkt, wvt, bq, bk, bv, mask, out)
    nc.compile()
    return nc


def _emit(nc, tc, xt, wqt, wkt, wvt, bq, bk, bv, mask, out):
    with (
        tc.tile_pool(name="singles", bufs=1) as singles,
        tc.tile_pool(name="persist", bufs=1) as persist,
        tc.tile_pool(name="wpool", bufs=1) as wpool,
        tc.tile_pool(name="attn", bufs=1) as attn,
        tc.tile_pool(name="psum", bufs=1, space="PSUM") as psum,
    ):
        # persistent activations (all bf16)
        xts = [persist.tile([128, S], BF16, name=f"xts{dt}", tag=f"xts{dt}") for dt in range(8)]
        qts = [persist.tile([128, S], BF16, name=f"qt{p}", tag=f"qt{p}") for p in range(NP)]
        kts = [persist.tile([128, S], BF16, name=f"kt{p}", tag=f"kt{p}") for p in range(NP)]
        vaug = [
            persist.tile([128, HL, DH + 1], BF16, name=f"vaug{t}", tag=f"vaug{t}")
            for t in range(ST)
        ]

        # DMA order is the startup critical path: wv first (V proj races
        # the xt stream), then xt, then small/late-needed tensors, then
        # QK weights (first used ~35us in).
        wv = wpool.tile([128, 8, O], BF16, name="wv", tag="wv")
        nc.sync.dma_start(out=wv[:, 0:1, :], in_=wvt[:, 0:1, :])
        nc.sync.dma_start(out=xts[0], in_=xt[0:128, :])
        nc.sync.dma_start(out=wv[:, 1:4, :], in_=wvt[:, 1:4, :])
        bv_bc = singles.tile([128, HL, DH], F32)
        nc.sync.dma_start(
            out=bv_bc, in_=bass.AP(tensor=bv.tensor, offset=0, ap=[[0, 128], [1, O]])
        )
        nc.sync.dma_start(out=xts[1], in_=xt[128:256, :])
        nc.sync.dma_start(out=wv[:, 4:8, :], in_=wvt[:, 4:8, :])
        for dt in range(2, 8):
            nc.sync.dma_start(out=xts[dt], in_=xt[dt * 128 : (dt + 1) * 128, :])
        ident = singles.tile([128, 128], F32)
        make_identity(nc, ident)
        wk = wpool.tile([128, NP, 8, 128], BF16, name="wk", tag="wk")
        nc.sync.dma_start(out=wk, in_=wkt)
        wq = wpool.tile([128, NP, 8, 128], BF16, name="wq", tag="wq")
        nc.sync.dma_start(out=wq, in_=wqt)
        wsl = {"k": wk, "q": wq}
        bq_sb = singles.tile([128, NP], F32)
        nc.sync.dma_start(out=bq_sb, in_=bq.rearrange("(t p) -> p t", p=128))
        bk_sb = singles.tile([128, NP], F32)
        nc.sync.dma_start(out=bk_sb, in_=bk.rearrange("(t p) -> p t", p=128))
        mask_sb = singles.tile([128, ST], F32)
        nc.sync.dma_start(out=mask_sb, in_=mask.rearrange("(t p) -> p t", p=128))
        # Schraudolph fast-exp constants for the DVE-offloaded slots:
        # exp(0.125*s + m) ~= bitcast_f32(int32(s*SCH_A + (m*SCH_M + SCH_B)))
        # with SCH_B tuned for minimax relative error (~+-3.5%).
        SCH_A = 0.125 * 1.4426950408889634 * 8388608.0
        SCH_M = 1.4426950408889634 * 8388608.0
        SCH_B = 127.0 * 8388608.0 - 297795.0
        mb_sb = singles.tile([128, ST], F32)
        nc.vector.tensor_scalar(
            out=mb_sb, in0=mask_sb, scalar1=SCH_M, scalar2=SCH_B,
            op0=mybir.AluOpType.mult, op1=mybir.AluOpType.add,
        )

        # vaug ones-columns: DVE is idle now, do them all up front
        for st in range(ST):
            nc.vector.memset(vaug[st][:, :, DH : DH + 1], 1.0)

        # ---- V projection: 2 passes x 8 s-tiles over all 8 PSUM banks,
        # dt-outer so pass 1 consumes xt chunks as the DMA delivers them.
        # Drains interleave into the dt=7 row so the pass handoff never
        # serializes on DVE.
        def v_pass(sb):  # sb = base s-tile (0 or 8)
            t01 = psum.tile([128, 2, HL, DH], F32, name=f"psv{sb}a", tag="sp0", bufs=1)
            t23 = psum.tile([128, 2, HL, DH], F32, name=f"psv{sb}b", tag="sp1", bufs=1)
            singles_ = [
                psum.tile([128, HL, DH], F32, name=f"psv{sb}_{i}", tag=t, bufs=1)
                for i, t in enumerate(("c0", "c1", "proj", "tp"))
            ]
            dsts = [t01[:, 0], t01[:, 1], t23[:, 0], t23[:, 1]] + singles_
            for dt in range(8):
                for g in range(8):
                    st = sb + g
                    nc.tensor.matmul(
                        dsts[g],
                        xts[dt][:, st * 128 : (st + 1) * 128],
                        wv[:, dt, :],
                        start=(dt == 0),
                        stop=(dt == 7),
                    )
                    if dt == 7:
                        va = vaug[st]
                        nc.vector.tensor_add(va[:, :, 0:DH], dsts[g], bv_bc)

        v_pass(0)
        v_pass(8)

        # ---- QK projection machinery ----
        qk_tag = [0]

        def emit_qk_chunk(which, p, c):
            w = wsl[which]
            dst = {"k": kts, "q": qts}[which][p]
            bias_sb = {"k": bk_sb, "q": bq_sb}[which]
            tag = ("proj", "tp")[qk_tag[0] % 2]
            qk_tag[0] += 1
            ps = psum.tile([128, 512], F32, name=f"ps{which}{p}_{c}", tag=tag, bufs=1)
            for dt in range(8):
                nc.tensor.matmul(
                    ps,
                    w[:, p, dt, :],
                    xts[dt][:, c * 512 : (c + 1) * 512],
                    start=(dt == 0),
                    stop=(dt == 7),
                )
            nc.vector.tensor_scalar_add(
                dst[:, c * 512 : (c + 1) * 512], ps, bias_sb[:, p : p + 1]
            )

        # Upfront: all of kt(p0) (scores at (p0,qc0) span every key chunk)
        # plus qt(p0,c0).
        for c in range(4):
            emit_qk_chunk("k", 0, c)
        emit_qk_chunk("q", 0, 0)

        # Remaining chunks drip-fed into attention slots.
        proj_chunks = [("q", 0, 1), ("q", 0, 2), ("q", 0, 3)]
        for p in range(1, NP):
            for c in range(4):
                proj_chunks.append(("k", p, c))
            for c in range(4):
                proj_chunks.append(("q", p, c))
        proj_cursor = [0, 0]  # chunk index, dt index

        def proj_mms_left():
            ci, dt = proj_cursor
            return (len(proj_chunks) - ci) * 8 - dt

        def emit_proj_mm():
            ci, dt = proj_cursor
            if ci >= len(proj_chunks):
                return False
            which, p, c = proj_chunks[ci]
            if dt == 0:
                emit_proj_mm.ps = psum.tile(
                    [128, 512], F32, name=f"ps{which}{p}_{c}", tag="proj", bufs=1
                )
            nc.tensor.matmul(
                emit_proj_mm.ps,
                wsl[which][:, p, dt, :],
                xts[dt][:, c * 512 : (c + 1) * 512],
                start=(dt == 0),
                stop=(dt == 7),
            )
            if dt == 7:
                dst = {"k": kts, "q": qts}[which][p]
                bias_sb = {"k": bk_sb, "q": bq_sb}[which]
                nc.vector.tensor_scalar_add(
                    dst[:, c * 512 : (c + 1) * 512], emit_proj_mm.ps,
                    bias_sb[:, p : p + 1],
                )
                proj_cursor[0] += 1
                proj_cursor[1] = 0
            else:
                proj_cursor[1] += 1
            return True

        # ---- attention: global software pipeline over 256 (p,qc,jt)
        # slots.  Slot g: scores(g) -> exp(g) -> [proj drip] -> ctx(g-2)
        # -> [drain step in slots 4..11].
        steps = [(p, qc, jt) for p in range(NP) for qc in range(QC) for jt in range(ST)]
        hist = {}
        Cs = {}
        drain_q = []

        def emit_scores(g, p, qc, jt):
            base = qc * 512
            sp = psum.tile([128, 2, 512], F32, name=f"sp{g}", tag=f"sp{g % 2}", bufs=1)
            for x in range(2):
                hp = slice(x * 64, x * 64 + 64)
                nc.tensor.matmul(
                    sp[:, x, :],
                    kts[p][hp, jt * 128 : (jt + 1) * 128],
                    qts[p][hp, base : base + 512],
                    start=True,
                    stop=True,
                )
            u = attn.tile([128, 2, 512], BF16, name=f"u{g}", tag=f"u{g % 4}", bufs=1)
            if jt == 13:
                # offload this slot's exp to the (otherwise idle) DVE via
                # Schraudolph's bit-trick so ACT catches up to the PE pace.
                iu = attn.tile(
                    [128, 2, 512], mybir.dt.int32, name=f"iu{g}", tag=f"iu{g % 2}",
                    bufs=1,
                )
                nc.vector.tensor_scalar(
                    out=iu, in0=sp, scalar1=SCH_A, scalar2=mb_sb[:, jt : jt + 1],
                    op0=mybir.AluOpType.mult, op1=mybir.AluOpType.add,
                )
                nc.vector.tensor_copy(out=u, in_=iu[:, :, :].bitcast(F32))
            else:
                nc.scalar.activation(
                    u, sp, EXP, bias=mask_sb[:, jt : jt + 1], scale=0.125
                )
            hist[g] = (u, p, qc, jt)

        def emit_ctx(g):
            u, p, qc, jt = hist.pop(g)
            if jt == 0:
                Cs[p, qc] = [
                    psum.tile(
                        [DH + 1, 512], F32, name=f"c{x}_{p}_{qc}", tag=f"c{x}", bufs=1
                    )
                    for x in range(2)
                ]
            C = Cs[p, qc]
            last = jt == ST - 1
            csb = []
            for x in range(2):
                nc.tensor.matmul(
                    C[x],
                    vaug[jt][:, 2 * p + x, :],
                    u[:, x, :],
                    start=(jt == 0),
                    stop=last,
                )
                if last:
                    cs = attn.tile(
                        [DH + 1, 512], F32, name=f"csb{p}_{qc}_{x}", tag=f"csb{x}",
                        bufs=2,
                    )
                    nc.vector.tensor_copy(out=cs, in_=C[x])
                    csb.append(cs)
            if last:
                Cs.pop((p, qc))
                ot = attn.tile(
                    [128, 4, 2, DH], F32, name=f"ot{p}_{qc}", tag="ot", bufs=2
                )
                drain_q.append({"csb": csb, "ot": ot, "p": p, "qc": qc, "i": 0})

        def emit_drain_step(tag="tp"):
            if not drain_q:
                return
            dr = drain_q[0]
            i = dr["i"]
            it, x = i // 2, i % 2
            tp_ = psum.tile(
                [128, DH + 1], F32, name=f"tp{dr['p']}_{dr['qc']}_{i}", tag=tag, bufs=1
            )
            nc.tensor.transpose(
                tp_,
                dr["csb"][x][:, it * 128 : (it + 1) * 128],
                ident[0 : DH + 1, 0 : DH + 1],
            )
            rc = attn.tile(
                [128, 1], F32, name=f"rc{dr['p']}_{dr['qc']}_{i}", tag="rc", bufs=4
            )
            nc.vector.reciprocal(rc, tp_[:, DH : DH + 1])
            nc.vector.tensor_scalar_mul(dr["ot"][:, it, x, :], tp_[:, 0:DH], rc)
            dr["i"] += 1
            if dr["i"] == 8:
                p, qc = dr["p"], dr["qc"]
                base = qc * 512
                nc.sync.dma_start(
                    out=out[
                        base : base + 512, 2 * p * DH : (2 * p + 2) * DH
                    ].rearrange("(i p) c -> p i c", p=128),
                    in_=dr["ot"],
                )
                drain_q.pop(0)

        NSLOT = len(steps)
        for g, (p, qc, jt) in enumerate(steps):
            emit_scores(g, p, qc, jt)
            left = proj_mms_left()
            if left > 0 and jt >= 2:
                quota = min(2, max(0, -(-left // max(1, (NSLOT - g)))))
                ci = proj_cursor[0]
                need_this_pair = 0
                for k in range(ci, len(proj_chunks)):
                    if proj_chunks[k][1] <= p + 1:
                        need_this_pair += 8
                    else:
                        break
                if need_this_pair:
                    need_this_pair -= proj_cursor[1]
                    pair_slots_left = 64 - (g % 64)
                    quota = min(2, max(quota, -(-need_this_pair // pair_slots_left)))
                for _ in range(quota):
                    emit_proj_mm()
            if g >= 3:
                emit_ctx(g - 3)
            if 4 <= jt <= 11:
                emit_drain_step()
        # pipeline tail: last three ctx slots, then the final drain fanned
        # out over the 4 now-free PSUM banks.
        emit_ctx(NSLOT - 3)
        emit_ctx(NSLOT - 2)
        emit_ctx(NSLOT - 1)
        k = 0
        while drain_q:
            emit_drain_step(tag=("proj", "tp", "c0", "c1")[k % 4])
            k += 1


def _make_in_maps(hidden_states, attention_mask, Wq, bq, Wk, bk, Wv, bv):
    bf = ml_dtypes.bfloat16

    def wqk_tiled(W, sl):
        # W.T shard [D, O] -> [128, NP, 8, 128]: row p holds
        # W.T[dt*128+p, pair*128:(pair+1)*128] at [p, pair, dt, :]
        wt = W[sl, :].T.astype(bf)  # [D, O]
        return np.ascontiguousarray(
            wt.reshape(8, 128, NP, 128).transpose(1, 2, 0, 3)
        )

    def wv_tiled(W, sl):
        wt = W[sl, :].T.astype(bf)  # [D, O]
        return np.ascontiguousarray(wt.reshape(8, 128, O).transpose(1, 0, 2))

    in_maps = []
    for c in range(8):
        b, hg = divmod(c, 2)
        sl = slice(hg * O, (hg + 1) * O)
        in_maps.append(
            {
                "xt": np.ascontiguousarray(hidden_states[b].T.astype(bf)),
                "wqt": wqk_tiled(Wq, sl),
                "wkt": wqk_tiled(Wk, sl),
                "wvt": wv_tiled(Wv, sl),
                "bq": np.ascontiguousarray(bq[sl]),
                "bk": np.ascontiguousarray(bk[sl]),
                "bv": np.ascontiguousarray(bv[sl]),
                "mask": np.ascontiguousarray(attention_mask[b, 0, 0, :]),
            }
        )
    return in_maps


def _gather(results):
    out = np.empty((B, S, D), dtype=np.float32)
    for c in range(8):
        b, hg = divmod(c, 2)
        out[b, :, hg * O : (hg + 1) * O] = results[c]["out"]
    return out


def kernel(hidden_states, attention_mask, Wq, bq, Wk, bk, Wv, bv, **run_kwargs):
    global _NC_CACHE
    args = [hidden_states, attention_mask, Wq, bq, Wk, bk, Wv, bv]
    args = [np.asarray(a, dtype=np.float32) for a in args]
    if _NC_CACHE is None:
        _NC_CACHE = build_nc()
    in_maps = _make_in_maps(*args)
    res = run_bass_kernel_spmd(_NC_CACHE, in_maps, core_ids=list(range(8)), **run_kwargs)
    kernel.last_result = res
    return _gather(res.results)
